# revision 1
# baseline (speedup 1.0000x reference)
"""Trainium2 Bass kernel for nn_ExemplarSoftmaxLoss (data-parallel over 8 cores).

Strategy:
  - Shard batch dim B (and the 3 B-row blocks of `outputs`) across 8 cores.
  - Per core, on device:
      * softmax part: per-row sum(exp(x)) via ScalarE Exp with row-accumulate
        (no max subtraction needed: |x| <= ~6 so exp is safely in fp32 range);
        label logits extracted on VectorE with a fused
        (iota == label) * x row-sum (scalar_tensor_tensor with accum_out).
      * distance part: exemplar rows gathered per 128-row block with
        [128,1]-offset indirect DMAs (the only HW-verified gather pattern),
        diffs (x + eps - y) on VectorE, squared-row-sums on ScalarE
        (Square w/ accumulate), sqrt + hinge logic on tiny [128,16] tiles.
      * outputs per-partition partial sums [128, 4].
  - Host: float64 reduction of the 8x[128,4] partials -> 4 scalar losses.
"""

import os
import sys

import numpy as np

for _p in ("/opt/trn_rl_repo",):
    if _p not in sys.path and os.path.isdir(_p):
        sys.path.insert(0, _p)

import concourse.bass as bass
import concourse.tile as tile
from concourse import bacc, mybir
from concourse._compat import with_exitstack
from concourse.bass_utils import run_bass_kernel_spmd

# If BASS_TRACE is set in the environment, run_bass_kernel_spmd imports
# antenv.axon_hooks, which this image lacks -- stub it so we degrade to
# an untraced run instead of crashing.
try:
    import antenv.axon_hooks  # noqa: F401
except ImportError:
    import types as _types

    _m = _types.ModuleType("antenv.axon_hooks")
    _m.get_axon_ntff_profile_hook = lambda: None
    _m.set_axon_ntff_profile_hook = lambda h: None
    sys.modules["antenv.axon_hooks"] = _m

# Problem constants (hardcoded per the harness contract).
B, D, C = 16384, 512, 1000
NCORES = 8
BS = B // NCORES  # 2048 batch rows per core
RS = 3 * BS  # 6144 softmax rows per core
P = 128
NB = BS // P  # 16 row-blocks in the distance phase
NR = RS // P  # 48 row-blocks in the softmax phase
NG = 4  # groups of 4 row-blocks in the distance phase
EPS = 1e-6
MARGIN2 = 0.2
LAMBDA = 1.0

f32 = mybir.dt.float32
i32 = mybir.dt.int32
Alu = mybir.AluOpType
Act = mybir.ActivationFunctionType
AX = mybir.AxisListType

LAST_RESULTS = None  # BassKernelResults of the most recent run (for test.py)


@with_exitstack
def _emit(ctx, tc, outs, ins):
    nc = tc.nc
    xo = ins["xout"]  # [RS, C]   f32 outputs shard (3 blocks concatenated)
    aa = ins["anc"]  # [BS, D]   f32
    pp = ins["pos"]  # [BS, D]   f32
    ng = ins["neg"]  # [BS, D]   f32
    ex = ins["exem"]  # [C, D]    f32 exemplar table
    la = ins["lab_a"]  # [P, NB]  i32  labels_anchor, row blk*128+p at [p, blk]
    ln = ins["lab_n"]  # [P, NB]  i32  labels_neg
    lf = ins["lab_f"]  # [P, NR]  f32  concat labels as f32, row rb*128+p at [p, rb]
    pd = outs["partials"]  # [P, 4]  f32

    sing = ctx.enter_context(tc.tile_pool(name="sing", bufs=1))
    xpool = ctx.enter_context(tc.tile_pool(name="xp", bufs=5))
    ejp = ctx.enter_context(tc.tile_pool(name="ejp", bufs=2, space="PSUM"))
    ljp = ctx.enter_context(tc.tile_pool(name="ljp", bufs=2))
    apnp = ctx.enter_context(tc.tile_pool(name="apnp", bufs=2))
    expool = ctx.enter_context(tc.tile_pool(name="expool", bufs=2))
    dfp = ctx.enter_context(tc.tile_pool(name="dfp", bufs=3))
    sqp = ctx.enter_context(tc.tile_pool(name="sqp", bufs=3, space="PSUM"))
    sqvp = ctx.enter_context(tc.tile_pool(name="sqvp", bufs=3))

    sums = sing.tile([P, NR], f32)  # per-row sum(exp(x))
    lbl = sing.tile([P, 32], f32)  # label logits: 0..15 fused A+B, 16..31 C
    d2a = sing.tile([P, NB * 3], f32)  # sq dists: dr1,dn1,dr2
    d2v = sing.tile([P, NB * 3], f32)  # sq dists: dn2,tp,tn
    la_t = sing.tile([P, NB], i32)
    ln_t = sing.tile([P, NB], i32)
    lf_t = sing.tile([P, NR], f32)
    iota2 = sing.tile([P, 2, C], f32)

    # small loads via SWDGE so the Sync HWDGE queue leads with the x-tile stream
    nc.gpsimd.dma_start(out=la_t[:], in_=la[:])
    nc.gpsimd.dma_start(out=ln_t[:], in_=ln[:])
    nc.gpsimd.dma_start(out=lf_t[:], in_=lf[:])
    for half in range(2):
        nc.gpsimd.iota(
            iota2[:, half, :],
            pattern=[[1, C]],
            base=0,
            channel_multiplier=0,
            allow_small_or_imprecise_dtypes=True,
        )

    def emit_gathers(g, exa, exn, b2s=range(4), exa_first=False):
        pairs_ = (
            [(exa, la_t, b2) for b2 in b2s] + [(exn, ln_t, b2) for b2 in b2s]
            if exa_first
            else [t for b2 in b2s for t in ((exa, la_t, b2), (exn, ln_t, b2))]
        )
        for dst, lab_t, b2 in pairs_:
            blk = 4 * g + b2
            nc.gpsimd.indirect_dma_start(
                out=dst[:, b2, :],
                out_offset=None,
                in_=ex[:],
                in_offset=bass.IndirectOffsetOnAxis(
                    ap=lab_t[:, blk : blk + 1], axis=0
                ),
            )

    def emit_apn_loads(g):
        at = apnp.tile([P, 4, D], f32, tag="at", name=f"at{g}")
        pt = apnp.tile([P, 4, D], f32, tag="pt", name=f"pt{g}")
        nt = apnp.tile([P, 4, D], f32, tag="nt", name=f"nt{g}")
        r0, r1 = g * 4 * P, (g + 1) * 4 * P
        nc.sync.dma_start(
            out=at[:], in_=aa[r0:r1, :].rearrange("(t p) d -> p t d", p=P)
        )
        nc.sync.dma_start(
            out=pt[:], in_=pp[r0:r1, :].rearrange("(t p) d -> p t d", p=P)
        )
        nc.sync.dma_start(
            out=nt[:], in_=ng[r0:r1, :].rearrange("(t p) d -> p t d", p=P)
        )
        return at, pt, nt

    # software-pipeline the exemplar gathers + anchor/pos/neg loads one group
    # ahead so SWDGE descriptor generation and DMA overlap compute
    ex_tiles = {
        0: (
            expool.tile([P, 4, D], f32, tag="exa", name="exa0"),
            expool.tile([P, 4, D], f32, tag="exn", name="exn0"),
        )
    }
    emit_gathers(0, *ex_tiles[0], exa_first=True)
    apn_tiles = {}

    # [3, 2048, 1000] view: third t, batch row r.  Thirds 0 and 1 share
    # labels_anchor row-for-row, so their x-tiles are loaded PAIRED
    # (block rb + block rb+16) and the label-logit extraction runs once
    # over both (we only ever need the SUM of label logits).
    xo3 = xo.rearrange("(t r) c -> t r c", t=3)

    def emit_xtile(step):
        xt = xpool.tile([P, 2, C], f32, tag="xt", name=f"xt{step}")
        if step < 16:  # paired tile: blocks (step, step+16) from thirds 0,1
            # per-block DMAs: exp on block b starts once its half arrives
            for b in range(2):
                nc.sync.dma_start(
                    out=xt[:, b, :],
                    in_=xo3[b, step * P : (step + 1) * P, :],
                )
            rbs = (step, step + 16)
            lbl_cols = (step,)
        else:  # plain tile: 2 consecutive blocks from third 2
            jj = step - 16
            for b in range(2):
                r0 = (jj * 2 + b) * P
                nc.sync.dma_start(
                    out=xt[:, b, :],
                    in_=xo3[2, r0 : r0 + P, :],
                )
            rbs = (32 + 2 * jj, 33 + 2 * jj)
            lbl_cols = (16 + 2 * jj, 17 + 2 * jj)
        for b, rb in enumerate(rbs):
            ej = ejp.tile([P, C], f32, tag="ej")
            nc.scalar.activation(
                out=ej[:],
                in_=xt[:, b, :],
                func=Act.Exp,
                accum_out=sums[:, rb : rb + 1],
            )
        if len(lbl_cols) == 1:  # fused: sum((iota==l)*x) over BOTH thirds
            lj = ljp.tile([P, 2, C], f32, tag="lj")
            nc.vector.scalar_tensor_tensor(
                out=lj[:],
                in0=iota2[:],
                scalar=lf_t[:, rbs[0] : rbs[0] + 1],
                in1=xt[:],
                op0=Alu.is_equal,
                op1=Alu.mult,
                accum_out=lbl[:, lbl_cols[0] : lbl_cols[0] + 1],
            )
        else:
            for b, (rb, col) in enumerate(zip(rbs, lbl_cols)):
                lj = ljp.tile([P, 2, C], f32, tag="lj")
                nc.vector.scalar_tensor_tensor(
                    out=lj[:, 0, :],
                    in0=iota2[:, 0, :],
                    scalar=lf_t[:, rb : rb + 1],
                    in1=xt[:, b, :],
                    op0=Alu.is_equal,
                    op1=Alu.mult,
                    accum_out=lbl[:, col : col + 1],
                )

    def emit_pair(g, pair):
        xs, ys, d2t, ci, on_act = pair
        df = dfp.tile([P, 4, D], f32, tag="df")
        # df = x - y.  (The reference's +EPS inside the norm shifts d^2
        # by ~2*EPS*|sum(diff)| ~ 1e-7 relative -- negligible.)
        nc.vector.tensor_tensor(out=df[:], in0=xs, in1=ys, op=Alu.subtract)
        for b2 in range(4):
            col = (4 * g + b2) * 3 + ci
            if on_act[b2] if isinstance(on_act, tuple) else on_act:
                sq = sqp.tile([P, D], f32, tag="sqa")
                nc.scalar.activation(
                    out=sq[:],
                    in_=df[:, b2, :],
                    func=Act.Square,
                    accum_out=d2t[:, col : col + 1],
                )
            else:
                sq = sqvp.tile([P, D], f32, tag="sqv")
                # (df * 1.0) * df with sum-accumulate == row-sum of df^2
                nc.vector.scalar_tensor_tensor(
                    out=sq[:],
                    in0=df[:, b2, :],
                    scalar=1.0,
                    in1=df[:, b2, :],
                    op0=Alu.mult,
                    op1=Alu.mult,
                    accum_out=d2t[:, col : col + 1],
                )

    for g in range(NG):
        exa, exn = ex_tiles.pop(g)
        # group 0 delays apn and its pair work so the ramp feeds x-tiles first
        pair_sched = {2: [2], 3: [3], 4: [4, 0], 5: [5, 1]} if g == 0 else None
        pairs = None
        # interleave 1 x-tile : 1 distance pair for smooth per-engine FIFOs
        for pi in range(6):
            emit_xtile(6 * g + pi)

            if g == 0 and pi == 1:
                apn_tiles[0] = emit_apn_loads(0)
            if pi == 2 and g + 1 < NG:
                ex_tiles[g + 1] = (
                    expool.tile([P, 4, D], f32, tag="exa", name=f"exa{g + 1}"),
                    expool.tile([P, 4, D], f32, tag="exn", name=f"exn{g + 1}"),
                )
            if pi == 3 and g + 1 < NG:
                # prefetch next group's apn mid-group (decongests the ramp)
                apn_tiles[g + 1] = emit_apn_loads(g + 1)
            if pi >= 2 and g + 1 < NG:
                # spread next group's gathers: 2 indirect DMAs per step
                emit_gathers(g + 1, *ex_tiles[g + 1], b2s=[pi - 2])

            if pairs is None and g in apn_tiles and (pair_sched is None or pi + 1 >= 2):
                at, pt, nt = apn_tiles.pop(g)
                # squares: ~72 on ScalarE, ~24 on VectorE (measured balance)
                pairs = (
                    (at[:], exa[:], d2a, 0, True),  # d_ref1  -> ScalarE
                    (nt[:], exa[:], d2a, 1, True),  # d_neg1  -> ScalarE
                    (at[:], exn[:], d2a, 2, True),  # d_ref2  -> ScalarE
                    (nt[:], exn[:], d2v, 0, True),  # d_neg2  -> ScalarE
                    (at[:], pt[:], d2v, 1, g != 3),  # tp -> 12/4
                    (at[:], nt[:], d2v, 2, False),  # tn    -> VectorE
                )
            for pj in (pair_sched.get(pi, []) if pair_sched else [pi]):
                emit_pair(g, pairs[pj])

    # ---- tail ----
    # sqrts first, Ln last: one table-set switch each instead of thrashing,
    # and the VectorE hinge work starts as early as possible
    part = sing.tile([P, 4], f32)
    dda = sing.tile([P, NB * 3], f32)
    ddv = sing.tile([P, NB * 3], f32)
    nc.scalar.activation(out=dda[:], in_=d2a[:], func=Act.Sqrt)
    nc.scalar.activation(out=ddv[:], in_=d2v[:], func=Act.Sqrt)
    logs = sing.tile([P, NR], f32)
    nc.scalar.activation(out=logs[:], in_=sums[:], func=Act.Ln)
    nc.vector.reduce_sum(out=part[:, 0:1], in_=logs[:], axis=AX.X)
    nc.vector.reduce_sum(out=part[:, 1:2], in_=lbl[:], axis=AX.X)

    dA = dda[:].rearrange("p (b k) -> p b k", k=3)
    dV = ddv[:].rearrange("p (b k) -> p b k", k=3)

    x1 = sing.tile([P, NB], f32)
    m1 = sing.tile([P, NB], f32)
    c1 = sing.tile([P, NB], f32)
    x2 = sing.tile([P, NB], f32)
    c2 = sing.tile([P, NB], f32)
    x3 = sing.tile([P, NB], f32)
    t3 = sing.tile([P, NB], f32)
    ca = sing.tile([P, 1], f32)
    cb = sing.tile([P, 1], f32)

    # c1 = (dr1 - dn1 > 0) ? (dr1 - dn1 + MARGIN2) : 0
    nc.vector.tensor_tensor(out=x1[:], in0=dA[:, :, 0], in1=dA[:, :, 1], op=Alu.subtract)
    nc.vector.tensor_scalar(
        out=m1[:], in0=x1[:], scalar1=0.0, scalar2=None, op0=Alu.is_gt
    )
    nc.vector.scalar_tensor_tensor(
        out=c1[:], in0=x1[:], scalar=MARGIN2, in1=m1[:],
        op0=Alu.add, op1=Alu.mult, accum_out=ca[:],
    )
    # c2 = relu(dn2 - dr2)
    nc.vector.tensor_tensor(out=x2[:], in0=dV[:, :, 0], in1=dA[:, :, 2], op=Alu.subtract)
    nc.vector.tensor_scalar(
        out=c2[:], in0=x2[:], scalar1=0.0, scalar2=None,
        op0=Alu.max, op1=Alu.add, accum_out=cb[:],
    )
    # t = relu(tp - tn)
    nc.vector.tensor_tensor(out=x3[:], in0=dV[:, :, 1], in1=dV[:, :, 2], op=Alu.subtract)
    nc.vector.tensor_scalar(
        out=t3[:], in0=x3[:], scalar1=0.0, scalar2=None,
        op0=Alu.max, op1=Alu.add, accum_out=part[:, 3:4],
    )
    nc.vector.tensor_tensor(out=part[:, 2:3], in0=ca[:], in1=cb[:], op=Alu.add)
    nc.sync.dma_start(out=pd[:], in_=part[:])


_COMPILED = None


def _build():
    global _COMPILED
    if _COMPILED is not None:
        return _COMPILED
    nc = bacc.Bacc(
        "TRN2",
        target_bir_lowering=False,
        debug=False,
        enable_asserts=False,
        num_devices=NCORES,
    )
    ins = {
        "xout": nc.dram_tensor("xout", [RS, C], f32, kind="ExternalInput").ap(),
        "anc": nc.dram_tensor("anc", [BS, D], f32, kind="ExternalInput").ap(),
        "pos": nc.dram_tensor("pos", [BS, D], f32, kind="ExternalInput").ap(),
        "neg": nc.dram_tensor("neg", [BS, D], f32, kind="ExternalInput").ap(),
        "exem": nc.dram_tensor("exem", [C, D], f32, kind="ExternalInput").ap(),
        "lab_a": nc.dram_tensor("lab_a", [P, NB], i32, kind="ExternalInput").ap(),
        "lab_n": nc.dram_tensor("lab_n", [P, NB], i32, kind="ExternalInput").ap(),
        "lab_f": nc.dram_tensor("lab_f", [P, NR], f32, kind="ExternalInput").ap(),
    }
    outs = {
        "partials": nc.dram_tensor("partials", [P, 4], f32, kind="ExternalOutput").ap()
    }
    with tile.TileContext(nc) as tc:
        _emit(tc, outs, ins)
    nc.compile()
    _COMPILED = nc
    return nc


def _in_maps(anchor, positive, negative, outputs, labels_anchor, labels_neg, exemplars):
    anchor = np.asarray(anchor, np.float32)
    positive = np.asarray(positive, np.float32)
    negative = np.asarray(negative, np.float32)
    outputs = np.asarray(outputs, np.float32)
    exemplars = np.ascontiguousarray(np.asarray(exemplars, np.float32))
    la_all = np.asarray(labels_anchor).astype(np.int64)
    ln_all = np.asarray(labels_neg).astype(np.int64)

    maps = []
    for k in range(NCORES):
        sl = slice(k * BS, (k + 1) * BS)
        la, ln = la_all[sl], ln_all[sl]
        xo = np.ascontiguousarray(
            np.concatenate(
                [
                    outputs[k * BS : (k + 1) * BS],
                    outputs[B + k * BS : B + (k + 1) * BS],
                    outputs[2 * B + k * BS : 2 * B + (k + 1) * BS],
                ],
                axis=0,
            )
        )
        labels_cat = np.concatenate([la, la, ln])
        maps.append(
            {
                "xout": xo,
                "anc": np.ascontiguousarray(anchor[sl]),
                "pos": np.ascontiguousarray(positive[sl]),
                "neg": np.ascontiguousarray(negative[sl]),
                "exem": exemplars,
                "lab_a": np.ascontiguousarray(la.reshape(NB, P).T.astype(np.int32)),
                "lab_n": np.ascontiguousarray(ln.reshape(NB, P).T.astype(np.int32)),
                "lab_f": np.ascontiguousarray(
                    labels_cat.reshape(NR, P).T.astype(np.float32)
                ),
            }
        )
    return maps


def _combine(results):
    S = np.zeros(4, dtype=np.float64)
    for r in results:
        S += r["partials"].astype(np.float64).sum(axis=0)
    loss_softmax = (S[0] - S[1]) / (3 * B)
    loss_center = S[2]
    loss_triplet = S[3]
    loss_total = loss_softmax + 0.01 * loss_center + LAMBDA * loss_triplet
    return (
        np.float32(loss_total),
        np.float32(loss_triplet),
        np.float32(loss_softmax),
        np.float32(loss_center),
    )


def kernel(anchor, positive, negative, outputs, labels_anchor, labels_neg, exemplars):
    global LAST_RESULTS
    nc = _build()
    maps = _in_maps(
        anchor, positive, negative, outputs, labels_anchor, labels_neg, exemplars
    )
    res = run_bass_kernel_spmd(nc, maps, core_ids=list(range(NCORES)))
    LAST_RESULTS = res
    return _combine(res.results)



# revision 6
# speedup vs baseline: 1.1010x; 1.1010x over previous
"""Trainium2 Bass kernel for nn_ExemplarSoftmaxLoss (data-parallel over 8 cores).

v2 strategy (vs. the v1 gather-heavy baseline):
  - Host-side: rows of each core's shard are PERMUTED (all reductions are
    permutation-invariant): distance rows + xout thirds 0/1 sorted by
    labels_anchor, xout third 2 sorted by labels_neg.  Sorted rows make
    each 128-row block's labels fall in a narrow window, so the label-logit
    extraction only scans a static W=256 column window (4x less DVE work).
    Window bases are computed from the data before compile (kernel builds
    lazily) and baked in as constants.
  - Exemplar rows are gathered with dma_gather (2048 rows per table in 4
    chunked calls, ~0.3us of Q7 emission per 512 rows vs ~1.5us per 128-row
    indirect DMA) from a host-cast bf16 table: halves gather HBM traffic.
  - anchor/pos/neg are loaded as bf16 via SWDGE cast-DMA (free cast), so
    the 6 distance diffs run in the DVE 2x bf16 mode.
  - squares (row-sum-of-squares) are split across ScalarE (Act.Square,
    ~0.8us/tile) / DVE (STT, ~0.69us) / GpSimd (STT, ~1us) to balance.
  - xout loaded in 2MB tiles (4 row-blocks) on the sync HWDGE queue.
  - Host: float64 reduction of the 8x[128,4] partials -> 4 scalar losses.
"""

import os
import sys

import numpy as np
import ml_dtypes

for _p in ("/opt/trn_rl_repo",):
    if _p not in sys.path and os.path.isdir(_p):
        sys.path.insert(0, _p)

import concourse.bass as bass
import concourse.tile as tile
from concourse import bacc, mybir
from concourse._compat import with_exitstack
from concourse.bass_utils import run_bass_kernel_spmd

# If BASS_TRACE is set in the environment, run_bass_kernel_spmd imports
# antenv.axon_hooks, which this image lacks -- stub it so we degrade to
# an untraced run instead of crashing.
try:
    import antenv.axon_hooks  # noqa: F401
except ImportError:
    import types as _types

    _m = _types.ModuleType("antenv.axon_hooks")
    _m.get_axon_ntff_profile_hook = lambda: None
    _m.set_axon_ntff_profile_hook = lambda h: None
    sys.modules["antenv.axon_hooks"] = _m

# Problem constants (hardcoded per the harness contract).
B, D, C = 16384, 512, 1000
NCORES = 8
BS = B // NCORES  # 2048 batch rows per core
RS = 3 * BS  # 6144 softmax rows per core
P = 128
NB = BS // P  # 16 row-blocks in the distance phase
NR = RS // P  # 48 row-blocks in the softmax phase
NG = 4  # groups of 4 row-blocks in the distance phase
XT = 4  # xout row-blocks per DMA tile
NXT = NR // XT  # 12 xout tiles
W = 256  # label extraction window width
EPS = 1e-6
MARGIN2 = 0.2
LAMBDA = 1.0

f32 = mybir.dt.float32
bf16 = mybir.dt.bfloat16
i16 = mybir.dt.int16
Alu = mybir.AluOpType
Act = mybir.ActivationFunctionType
AX = mybir.AxisListType

LAST_RESULTS = None  # BassKernelResults of the most recent run (for test.py)

# square-engine assignment per (pair ci 0..5, block-in-group b2 0..3):
# 'S' ScalarE Act.Square, 'V' DVE STT.  10 S / 14 V per group -> 40 / 56.
# (GpSimd can't run TensorScalarPtr -- the ISA check rejects it.)
SQ_ENGINE = [
    "SVSV",  # d_ref1
    "VSVS",  # d_neg1
    "SVSV",  # d_ref2
    "VSVS",  # d_neg2
    "SVVV",  # tp
    "VVSV",  # tn
]


@with_exitstack
def _emit(ctx, tc, outs, ins, bases):
    nc = tc.nc
    xo = ins["xout"]  # [RS, C] f32 (3 thirds, host-permuted)
    aa = ins["anc"]  # [BS, D] f32 (sorted by la)
    pp = ins["pos"]  # [BS, D] f32
    ng = ins["neg"]  # [BS, D] f32
    ex = ins["exem"]  # [C, D]  bf16 exemplar table
    ia = ins["idxa"]  # [128, 128] i16 wrapped gather idx (= sorted la)
    in_ = ins["idxn"]  # [128, 128] i16 wrapped gather idx (= ln[perm_a])
    lsh = ins["labsh"]  # [P, NR] f32 label - window_base per block
    pd = outs["partials"]  # [P, 4] f32

    sing = ctx.enter_context(tc.tile_pool(name="sing", bufs=1))
    xpool = ctx.enter_context(tc.tile_pool(name="xp", bufs=3))
    ejp = ctx.enter_context(tc.tile_pool(name="ejp", bufs=2, space="PSUM"))
    sqp = ctx.enter_context(tc.tile_pool(name="sqp", bufs=3, space="PSUM"))
    ljp = ctx.enter_context(tc.tile_pool(name="ljp", bufs=2))
    sqvp = ctx.enter_context(tc.tile_pool(name="sqvp", bufs=3))
    dfp = ctx.enter_context(tc.tile_pool(name="dfp", bufs=3))

    sums = sing.tile([P, NR], f32)  # per-row sum(exp(x))
    lbl = sing.tile([P, NR], f32)  # label logits per block
    d2a = sing.tile([P, NB * 3], f32)  # sq dists: dr1,dn1,dr2
    d2v = sing.tile([P, NB * 3], f32)  # sq dists: dn2,tp,tn
    iota_w = sing.tile([P, W], f32)
    lsh_t = sing.tile([P, NR], f32)
    ia_t = sing.tile([128, 128], i16)
    in_t = sing.tile([128, 128], i16)
    at = sing.tile([P, NB, D], bf16)
    pt = sing.tile([P, NB, D], bf16)
    nt = sing.tile([P, NB, D], bf16)
    exa = sing.tile([P, NB, D], bf16)
    exn = sing.tile([P, NB, D], bf16)

    # small loads via SWDGE so the Sync HWDGE queue leads with the x-tile stream
    nc.gpsimd.dma_start(out=lsh_t[:], in_=lsh[:])
    nc.gpsimd.dma_start(out=ia_t[:], in_=ia[:])
    nc.gpsimd.dma_start(out=in_t[:], in_=in_[:])
    nc.gpsimd.iota(
        iota_w[:],
        pattern=[[1, W]],
        base=0,
        channel_multiplier=0,
        allow_small_or_imprecise_dtypes=True,
    )

    xo3 = xo.rearrange("(t r) c -> t r c", t=3)
    xt_tiles = {}

    def emit_xload(s):
        # tile s covers third t = s // 4, blocks 4*(s%4) .. 4*(s%4)+3
        t, j4 = s // XT, (s % XT) * XT
        xt = xpool.tile([P, XT, C], f32, tag="xt", name=f"xt{s}")
        nc.sync.dma_start(
            out=xt[:],
            in_=xo3[t, j4 * P : (j4 + XT) * P, :].rearrange(
                "(t p) c -> p t c", p=P
            ),
        )
        xt_tiles[s] = xt

    def emit_xcompute(s):
        xt = xt_tiles.pop(s)
        t, j4 = s // XT, (s % XT) * XT
        for b in range(XT):
            j = j4 + b  # block index within the third
            col = 16 * t + j
            ej = ejp.tile([P, C], f32, tag="ej")
            nc.scalar.activation(
                out=ej[:],
                in_=xt[:, b, :],
                func=Act.Exp,
                accum_out=sums[:, col : col + 1],
            )
            base = bases[j]
            lj = ljp.tile([P, W], f32, tag="lj")
            nc.vector.scalar_tensor_tensor(
                out=lj[:],
                in0=iota_w[:],
                scalar=lsh_t[:, col : col + 1],
                in1=xt[:, b, base : base + W],
                op0=Alu.is_equal,
                op1=Alu.mult,
                accum_out=lbl[:, col : col + 1],
            )

    def emit_apn_loads(g):
        r0, r1 = g * 4 * P, (g + 1) * 4 * P
        for dst, src in ((at, aa), (pt, pp), (nt, ng)):
            nc.gpsimd.dma_start(
                out=dst[:, 4 * g : 4 * g + 4, :],
                in_=src[r0:r1, :].rearrange("(t p) d -> p t d", p=P),
            )

    def emit_gathers(g):
        for dst, idx in ((exa, ia_t), (exn, in_t)):
            nc.gpsimd.dma_gather(
                dst[:, 4 * g : 4 * g + 4, :],
                ex[:],
                idx[:, 32 * g : 32 * g + 32],
                512,
                512,
                D,
            )

    def emit_sq(df, b2, d2t, ci, g, eng):
        col = (4 * g + b2) * 3 + ci
        if eng == "S":
            sq = sqp.tile([P, D], f32, tag="sqa")
            nc.scalar.activation(
                out=sq[:],
                in_=df[:, b2, :],
                func=Act.Square,
                accum_out=d2t[:, col : col + 1],
            )
        else:
            sq = sqvp.tile([P, D], bf16, tag="sqv")
            nc.vector.scalar_tensor_tensor(
                out=sq[:],
                in0=df[:, b2, :],
                scalar=1.0,
                in1=df[:, b2, :],
                op0=Alu.mult,
                op1=Alu.mult,
                accum_out=d2t[:, col : col + 1],
            )

    def emit_group(g):
        s4 = slice(4 * g, 4 * g + 4)
        pairs = (
            (at, exa, d2a, 0),  # d_ref1
            (nt, exa, d2a, 1),  # d_neg1
            (at, exn, d2a, 2),  # d_ref2
            (nt, exn, d2v, 0),  # d_neg2
            (at, pt, d2v, 1),  # tp
            (at, nt, d2v, 2),  # tn
        )
        for pi, (xs, ys, d2t, ci) in enumerate(pairs):
            df = dfp.tile([P, 4, D], bf16, tag="df")
            nc.vector.tensor_tensor(
                out=df[:], in0=xs[:, s4, :], in1=ys[:, s4, :], op=Alu.subtract
            )
            for b2 in range(4):
                emit_sq(df, b2, d2t, ci, g, SQ_ENGINE[pi][b2])

    # ---- main schedule ----
    emit_xload(0)
    emit_apn_loads(0)
    emit_gathers(0)
    emit_xload(1)
    emit_apn_loads(1)
    emit_gathers(1)

    # distance group g computes after xout tile 2g+1; its loads go out early
    for s in range(NXT):
        if s + 2 < NXT:
            emit_xload(s + 2)
        if s == 2:
            emit_apn_loads(2)
            emit_gathers(2)
        if s == 4:
            emit_apn_loads(3)
            emit_gathers(3)
        emit_xcompute(s)
        if s in (3, 5, 7, 9):
            emit_group((s - 3) // 2)

    # ---- tail ----
    # sqrts first, Ln last: one table-set switch each instead of thrashing,
    # and the VectorE hinge work starts as early as possible
    part = sing.tile([P, 4], f32)
    dda = sing.tile([P, NB * 3], f32)
    ddv = sing.tile([P, NB * 3], f32)
    nc.scalar.activation(out=dda[:], in_=d2a[:], func=Act.Sqrt)
    nc.scalar.activation(out=ddv[:], in_=d2v[:], func=Act.Sqrt)
    logs = sing.tile([P, NR], f32)
    nc.scalar.activation(out=logs[:], in_=sums[:], func=Act.Ln)
    nc.vector.reduce_sum(out=part[:, 0:1], in_=logs[:], axis=AX.X)
    nc.vector.reduce_sum(out=part[:, 1:2], in_=lbl[:], axis=AX.X)

    dA = dda[:].rearrange("p (b k) -> p b k", k=3)
    dV = ddv[:].rearrange("p (b k) -> p b k", k=3)

    x1 = sing.tile([P, NB], f32)
    m1 = sing.tile([P, NB], f32)
    c1 = sing.tile([P, NB], f32)
    x2 = sing.tile([P, NB], f32)
    c2 = sing.tile([P, NB], f32)
    x3 = sing.tile([P, NB], f32)
    t3 = sing.tile([P, NB], f32)
    ca = sing.tile([P, 1], f32)
    cb = sing.tile([P, 1], f32)

    # c1 = (dr1 - dn1 > 0) ? (dr1 - dn1 + MARGIN2) : 0
    nc.vector.tensor_tensor(out=x1[:], in0=dA[:, :, 0], in1=dA[:, :, 1], op=Alu.subtract)
    nc.vector.tensor_scalar(
        out=m1[:], in0=x1[:], scalar1=0.0, scalar2=None, op0=Alu.is_gt
    )
    nc.vector.scalar_tensor_tensor(
        out=c1[:], in0=x1[:], scalar=MARGIN2, in1=m1[:],
        op0=Alu.add, op1=Alu.mult, accum_out=ca[:],
    )
    # c2 = relu(dn2 - dr2)
    nc.vector.tensor_tensor(out=x2[:], in0=dV[:, :, 0], in1=dA[:, :, 2], op=Alu.subtract)
    nc.vector.tensor_scalar(
        out=c2[:], in0=x2[:], scalar1=0.0, scalar2=None,
        op0=Alu.max, op1=Alu.add, accum_out=cb[:],
    )
    # t = relu(tp - tn)
    nc.vector.tensor_tensor(out=x3[:], in0=dV[:, :, 1], in1=dV[:, :, 2], op=Alu.subtract)
    nc.vector.tensor_scalar(
        out=t3[:], in0=x3[:], scalar1=0.0, scalar2=None,
        op0=Alu.max, op1=Alu.add, accum_out=part[:, 3:4],
    )
    nc.vector.tensor_tensor(out=part[:, 2:3], in0=ca[:], in1=cb[:], op=Alu.add)
    nc.sync.dma_start(out=pd[:], in_=part[:])


_COMPILED = {}


def _build(bases):
    key = tuple(bases)
    if key in _COMPILED:
        return _COMPILED[key]
    nc = bacc.Bacc(
        "TRN2",
        target_bir_lowering=False,
        debug=False,
        enable_asserts=False,
        num_devices=NCORES,
    )
    ins = {
        "xout": nc.dram_tensor("xout", [RS, C], f32, kind="ExternalInput").ap(),
        "anc": nc.dram_tensor("anc", [BS, D], f32, kind="ExternalInput").ap(),
        "pos": nc.dram_tensor("pos", [BS, D], f32, kind="ExternalInput").ap(),
        "neg": nc.dram_tensor("neg", [BS, D], f32, kind="ExternalInput").ap(),
        "exem": nc.dram_tensor("exem", [C, D], bf16, kind="ExternalInput").ap(),
        "idxa": nc.dram_tensor("idxa", [128, 128], i16, kind="ExternalInput").ap(),
        "idxn": nc.dram_tensor("idxn", [128, 128], i16, kind="ExternalInput").ap(),
        "labsh": nc.dram_tensor("labsh", [P, NR], f32, kind="ExternalInput").ap(),
    }
    outs = {
        "partials": nc.dram_tensor("partials", [P, 4], f32, kind="ExternalOutput").ap()
    }
    with tile.TileContext(nc) as tc:
        _emit(tc, outs, ins, bases)
    nc.compile()
    _COMPILED[key] = nc
    return nc


def _wrap_idx(v):
    # dma_gather index layout: idx i at [i % 16, i // 16], replicated to
    # each 16-partition group (one per Q7 core).
    w = np.asarray(v, np.int16).reshape(128, 16).T  # [16, 128]
    return np.ascontiguousarray(np.tile(w, (8, 1)))  # [128, 128]


def _prep(anchor, positive, negative, outputs, labels_anchor, labels_neg, exemplars):
    anchor = np.asarray(anchor, np.float32)
    positive = np.asarray(positive, np.float32)
    negative = np.asarray(negative, np.float32)
    outputs = np.asarray(outputs, np.float32)
    ex16 = np.ascontiguousarray(
        np.asarray(exemplars, np.float32).astype(ml_dtypes.bfloat16)
    )
    la_all = np.asarray(labels_anchor).astype(np.int64)
    ln_all = np.asarray(labels_neg).astype(np.int64)

    cores = []
    lo = np.full(NB, C, np.int64)
    hi = np.full(NB, -1, np.int64)
    for k in range(NCORES):
        sl = slice(k * BS, (k + 1) * BS)
        la, ln = la_all[sl], ln_all[sl]
        pa = np.argsort(la, kind="stable")
        pn = np.argsort(ln, kind="stable")
        la_s, ln_s = la[pa], ln[pn]
        for v in (la_s, ln_s):
            vb = v.reshape(NB, P)
            np.minimum(lo, vb.min(axis=1), out=lo)
            np.maximum(hi, vb.max(axis=1), out=hi)
        cores.append((k, sl, pa, pn, la_s, ln_s))

    span = hi - lo
    assert (span < W).all(), f"label window overflow: {span.max()} >= {W}"
    bases = np.minimum(np.minimum(lo, C - W), hi - W + 1)
    bases = np.maximum(bases, 0).astype(np.int64)
    assert ((bases <= lo) & (bases + W > hi)).all()

    maps = []
    for k, sl, pa, pn, la_s, ln_s in cores:
        ln_pa = ln_all[sl][pa]
        xo = np.ascontiguousarray(
            np.concatenate(
                [
                    outputs[k * BS : (k + 1) * BS][pa],
                    outputs[B + k * BS : B + (k + 1) * BS][pa],
                    outputs[2 * B + k * BS : 2 * B + (k + 1) * BS][pn],
                ],
                axis=0,
            )
        )
        labsh = np.empty((P, NR), np.float32)
        for t, v in enumerate((la_s, la_s, ln_s)):
            labsh[:, 16 * t : 16 * t + 16] = (
                (v.reshape(NB, P) - bases[:, None]).T.astype(np.float32)
            )
        maps.append(
            {
                "xout": xo,
                "anc": np.ascontiguousarray(anchor[sl][pa]),
                "pos": np.ascontiguousarray(positive[sl][pa]),
                "neg": np.ascontiguousarray(negative[sl][pa]),
                "exem": ex16,
                "idxa": _wrap_idx(la_s),
                "idxn": _wrap_idx(ln_pa),
                "labsh": np.ascontiguousarray(labsh),
            }
        )
    return maps, tuple(int(b) for b in bases)


def _combine(results):
    S = np.zeros(4, dtype=np.float64)
    for r in results:
        S += r["partials"].astype(np.float64).sum(axis=0)
    loss_softmax = (S[0] - S[1]) / (3 * B)
    loss_center = S[2]
    loss_triplet = S[3]
    loss_total = loss_softmax + 0.01 * loss_center + LAMBDA * loss_triplet
    return (
        np.float32(loss_total),
        np.float32(loss_triplet),
        np.float32(loss_softmax),
        np.float32(loss_center),
    )


def kernel(anchor, positive, negative, outputs, labels_anchor, labels_neg, exemplars):
    global LAST_RESULTS
    maps, bases = _prep(
        anchor, positive, negative, outputs, labels_anchor, labels_neg, exemplars
    )
    nc = _build(bases)
    res = run_bass_kernel_spmd(nc, maps, core_ids=list(range(NCORES)))
    LAST_RESULTS = res
    return _combine(res.results)


# revision 7
# speedup vs baseline: 1.2372x; 1.1237x over previous
"""Trainium2 Bass kernel for nn_ExemplarSoftmaxLoss (data-parallel over 8 cores).

v3 strategy:
  - Host-side: rows of each core's shard are PERMUTED (all reductions are
    permutation-invariant): distance rows + xout thirds 0/1 sorted by
    labels_anchor, xout third 2 sorted by labels_neg.  Sorted rows make
    each 128-row block's labels fall in a narrow window, so the label-logit
    extraction only scans a static W-column window (5x less DVE work).
    Window bases/width are computed from the data before compile (kernel
    builds lazily) and baked in as constants.
  - All bulk inputs are uploaded as bf16 (the 2e-2 rel-err budget makes
    mixed precision the right kernel design): halves HBM traffic to
    ~22.8 MB/core (~64 us of DMA) and enables the DVE 2x bf16 mode for
    the distance diffs.  All arithmetic still happens on device.
  - Exemplar rows gathered with chunked dma_gather (512 rows/call) from
    the bf16 table (~5 us of Q7 emission per call, vs ~1.5 us per
    128-row indirect DMA).
  - squares split ScalarE (Act.Square accum) / DVE (STT) to balance.
  - Host: float64 reduction of the 8x[128,4] partials -> 4 scalar losses.
"""

import os
import sys

import numpy as np
import ml_dtypes

for _p in ("/opt/trn_rl_repo",):
    if _p not in sys.path and os.path.isdir(_p):
        sys.path.insert(0, _p)

import concourse.bass as bass
import concourse.tile as tile
from concourse import bacc, mybir
from concourse._compat import with_exitstack
from concourse.bass_utils import run_bass_kernel_spmd

# If BASS_TRACE is set in the environment, run_bass_kernel_spmd imports
# antenv.axon_hooks, which this image lacks -- stub it so we degrade to
# an untraced run instead of crashing.
try:
    import antenv.axon_hooks  # noqa: F401
except ImportError:
    import types as _types

    _m = _types.ModuleType("antenv.axon_hooks")
    _m.get_axon_ntff_profile_hook = lambda: None
    _m.set_axon_ntff_profile_hook = lambda h: None
    sys.modules["antenv.axon_hooks"] = _m

# Problem constants (hardcoded per the harness contract).
B, D, C = 16384, 512, 1000
NCORES = 8
BS = B // NCORES  # 2048 batch rows per core
RS = 3 * BS  # 6144 softmax rows per core
P = 128
NB = BS // P  # 16 row-blocks in the distance phase
NR = RS // P  # 48 row-blocks in the softmax phase
NG = 4  # groups of 4 row-blocks in the distance phase
EPS = 1e-6
MARGIN2 = 0.2
LAMBDA = 1.0

# xout DMA tiles: (third, first block, n blocks).  2-block head tiles give
# the exp stream an early start; the rest are 4-block (1MB bf16) tiles.
TILE_SHAPES = [2, 2, 4, 4, 4]
TILES = [
    (t, sum(TILE_SHAPES[:i]), TILE_SHAPES[i])
    for t in range(3)
    for i in range(len(TILE_SHAPES))
]
NXT = len(TILES)  # 15

f32 = mybir.dt.float32
bf16 = mybir.dt.bfloat16
i16 = mybir.dt.int16
Alu = mybir.AluOpType
Act = mybir.ActivationFunctionType
AX = mybir.AxisListType

LAST_RESULTS = None  # BassKernelResults of the most recent run (for test.py)

# square-engine assignment per (pair ci 0..5, block-in-group b2 0..3):
# 'S' ScalarE Act.Square, 'V' DVE STT.  8 S / 16 V per group -> 32 / 64.
SQ_ENGINE = [
    "SVSV",  # d_ref1
    "VSVS",  # d_neg1
    "SVVV",  # d_ref2
    "VSVV",  # d_neg2
    "SVVV",  # tp
    "VVSV",  # tn
]


@with_exitstack
def _emit(ctx, tc, outs, ins, bases, W):
    nc = tc.nc
    xo = ins["xout"]  # [RS, C] bf16 (3 thirds, host-permuted)
    aa = ins["anc"]  # [BS, D] bf16 (sorted by la)
    pp = ins["pos"]  # [BS, D] bf16
    ng = ins["neg"]  # [BS, D] bf16
    ex = ins["exem"]  # [C, D]  bf16 exemplar table
    ia = ins["idxa"]  # [128, 128] i16 wrapped gather idx (= sorted la)
    in_ = ins["idxn"]  # [128, 128] i16 wrapped gather idx (= ln[perm_a])
    lsh = ins["labsh"]  # [P, NR] f32 label - window_base per block
    pd = outs["partials"]  # [P, 4] f32

    sing = ctx.enter_context(tc.tile_pool(name="sing", bufs=1))
    xpool = ctx.enter_context(tc.tile_pool(name="xp", bufs=3))
    ejp = ctx.enter_context(tc.tile_pool(name="ejp", bufs=2, space="PSUM"))
    sqp = ctx.enter_context(tc.tile_pool(name="sqp", bufs=3, space="PSUM"))
    ljp = ctx.enter_context(tc.tile_pool(name="ljp", bufs=2))
    sqvp = ctx.enter_context(tc.tile_pool(name="sqvp", bufs=3))
    dfp = ctx.enter_context(tc.tile_pool(name="dfp", bufs=3))

    sums = sing.tile([P, NR], f32)  # per-row sum(exp(x))
    lbl = sing.tile([P, NR], f32)  # label logits per block
    d2a = sing.tile([P, NB * 3], f32)  # sq dists: dr1,dn1,dr2
    d2v = sing.tile([P, NB * 3], f32)  # sq dists: dn2,tp,tn
    iota_w = sing.tile([P, W], f32)
    lsh_t = sing.tile([P, NR], f32)
    ia_t = sing.tile([128, 128], i16)
    in_t = sing.tile([128, 128], i16)
    at = sing.tile([P, NB, D], bf16)
    pt = sing.tile([P, NB, D], bf16)
    nt = sing.tile([P, NB, D], bf16)
    exa = sing.tile([P, NB, D], bf16)
    exn = sing.tile([P, NB, D], bf16)

    # small loads via SWDGE so the Sync HWDGE queue leads with the x-tile stream
    nc.gpsimd.dma_start(out=lsh_t[:], in_=lsh[:])
    nc.gpsimd.dma_start(out=ia_t[:], in_=ia[:])
    nc.gpsimd.dma_start(out=in_t[:], in_=in_[:])
    nc.gpsimd.iota(
        iota_w[:],
        pattern=[[1, W]],
        base=0,
        channel_multiplier=0,
        allow_small_or_imprecise_dtypes=True,
    )

    xo3 = xo.rearrange("(t r) c -> t r c", t=3)
    xt_tiles = {}

    def emit_xload(s):
        t, j0, nb = TILES[s]
        xt = xpool.tile([P, nb, C], bf16, tag="xt", name=f"xt{s}")
        nc.sync.dma_start(
            out=xt[:],
            in_=xo3[t, j0 * P : (j0 + nb) * P, :].rearrange(
                "(t p) c -> p t c", p=P
            ),
        )
        xt_tiles[s] = xt

    def emit_xcompute(s):
        xt = xt_tiles.pop(s)
        t, j0, nb = TILES[s]
        for b in range(nb):
            j = j0 + b  # block index within the third
            col = 16 * t + j
            ej = ejp.tile([P, C], f32, tag="ej")
            nc.scalar.activation(
                out=ej[:],
                in_=xt[:, b, :],
                func=Act.Exp,
                accum_out=sums[:, col : col + 1],
            )
            base = bases[j]
            lj = ljp.tile([P, W], f32, tag="lj")
            nc.vector.scalar_tensor_tensor(
                out=lj[:],
                in0=iota_w[:],
                scalar=lsh_t[:, col : col + 1],
                in1=xt[:, b, base : base + W],
                op0=Alu.is_equal,
                op1=Alu.mult,
                accum_out=lbl[:, col : col + 1],
            )

    def emit_apn_loads(g):
        r0, r1 = g * 4 * P, (g + 1) * 4 * P
        for dst, src in ((at, aa), (pt, pp), (nt, ng)):
            nc.sync.dma_start(
                out=dst[:, 4 * g : 4 * g + 4, :],
                in_=src[r0:r1, :].rearrange("(t p) d -> p t d", p=P),
            )

    def emit_gathers(g):
        for dst, idx in ((exa, ia_t), (exn, in_t)):
            nc.gpsimd.dma_gather(
                dst[:, 4 * g : 4 * g + 4, :],
                ex[:],
                idx[:, 32 * g : 32 * g + 32],
                512,
                512,
                D,
            )

    def emit_sq(df, b2, d2t, ci, g, eng):
        col = (4 * g + b2) * 3 + ci
        if eng == "S":
            sq = sqp.tile([P, D], f32, tag="sqa")
            nc.scalar.activation(
                out=sq[:],
                in_=df[:, b2, :],
                func=Act.Square,
                accum_out=d2t[:, col : col + 1],
            )
        else:
            sq = sqvp.tile([P, D], bf16, tag="sqv")
            nc.vector.scalar_tensor_tensor(
                out=sq[:],
                in0=df[:, b2, :],
                scalar=1.0,
                in1=df[:, b2, :],
                op0=Alu.mult,
                op1=Alu.mult,
                accum_out=d2t[:, col : col + 1],
            )

    def emit_group(g):
        s4 = slice(4 * g, 4 * g + 4)
        pairs = (
            (at, exa, d2a, 0),  # d_ref1
            (nt, exa, d2a, 1),  # d_neg1
            (at, exn, d2a, 2),  # d_ref2
            (nt, exn, d2v, 0),  # d_neg2
            (at, pt, d2v, 1),  # tp
            (at, nt, d2v, 2),  # tn
        )
        for pi, (xs, ys, d2t, ci) in enumerate(pairs):
            df = dfp.tile([P, 4, D], bf16, tag="df")
            nc.vector.tensor_tensor(
                out=df[:], in0=xs[:, s4, :], in1=ys[:, s4, :], op=Alu.subtract
            )
            for b2 in range(4):
                emit_sq(df, b2, d2t, ci, g, SQ_ENGINE[pi][b2])

    # ---- main schedule ----
    emit_xload(0)
    emit_xload(1)
    emit_apn_loads(0)
    emit_gathers(0)
    emit_xload(2)
    emit_apn_loads(1)
    emit_gathers(1)

    for s in range(NXT):
        if s + 3 < NXT:
            emit_xload(s + 3)
        if s == 2:
            emit_apn_loads(2)
            emit_gathers(2)
        if s == 4:
            emit_apn_loads(3)
            emit_gathers(3)
        emit_xcompute(s)
        if s in (4, 6, 8, 10):
            emit_group((s - 4) // 2)

    # ---- tail ----
    # sqrts first, Ln last: one table-set switch each instead of thrashing,
    # and the VectorE hinge work starts as early as possible
    part = sing.tile([P, 4], f32)
    dda = sing.tile([P, NB * 3], f32)
    ddv = sing.tile([P, NB * 3], f32)
    nc.scalar.activation(out=dda[:], in_=d2a[:], func=Act.Sqrt)
    nc.scalar.activation(out=ddv[:], in_=d2v[:], func=Act.Sqrt)
    logs = sing.tile([P, NR], f32)
    nc.scalar.activation(out=logs[:], in_=sums[:], func=Act.Ln)
    nc.vector.reduce_sum(out=part[:, 0:1], in_=logs[:], axis=AX.X)
    nc.vector.reduce_sum(out=part[:, 1:2], in_=lbl[:], axis=AX.X)

    dA = dda[:].rearrange("p (b k) -> p b k", k=3)
    dV = ddv[:].rearrange("p (b k) -> p b k", k=3)

    x1 = sing.tile([P, NB], f32)
    m1 = sing.tile([P, NB], f32)
    c1 = sing.tile([P, NB], f32)
    x2 = sing.tile([P, NB], f32)
    c2 = sing.tile([P, NB], f32)
    x3 = sing.tile([P, NB], f32)
    t3 = sing.tile([P, NB], f32)
    ca = sing.tile([P, 1], f32)
    cb = sing.tile([P, 1], f32)

    # c1 = (dr1 - dn1 > 0) ? (dr1 - dn1 + MARGIN2) : 0
    nc.vector.tensor_tensor(out=x1[:], in0=dA[:, :, 0], in1=dA[:, :, 1], op=Alu.subtract)
    nc.vector.tensor_scalar(
        out=m1[:], in0=x1[:], scalar1=0.0, scalar2=None, op0=Alu.is_gt
    )
    nc.vector.scalar_tensor_tensor(
        out=c1[:], in0=x1[:], scalar=MARGIN2, in1=m1[:],
        op0=Alu.add, op1=Alu.mult, accum_out=ca[:],
    )
    # c2 = relu(dn2 - dr2)
    nc.vector.tensor_tensor(out=x2[:], in0=dV[:, :, 0], in1=dA[:, :, 2], op=Alu.subtract)
    nc.vector.tensor_scalar(
        out=c2[:], in0=x2[:], scalar1=0.0, scalar2=None,
        op0=Alu.max, op1=Alu.add, accum_out=cb[:],
    )
    # t = relu(tp - tn)
    nc.vector.tensor_tensor(out=x3[:], in0=dV[:, :, 1], in1=dV[:, :, 2], op=Alu.subtract)
    nc.vector.tensor_scalar(
        out=t3[:], in0=x3[:], scalar1=0.0, scalar2=None,
        op0=Alu.max, op1=Alu.add, accum_out=part[:, 3:4],
    )
    nc.vector.tensor_tensor(out=part[:, 2:3], in0=ca[:], in1=cb[:], op=Alu.add)
    nc.sync.dma_start(out=pd[:], in_=part[:])


_COMPILED = {}


def _build(bases, W):
    key = (tuple(bases), W)
    if key in _COMPILED:
        return _COMPILED[key]
    nc = bacc.Bacc(
        "TRN2",
        target_bir_lowering=False,
        debug=False,
        enable_asserts=False,
        num_devices=NCORES,
    )
    ins = {
        "xout": nc.dram_tensor("xout", [RS, C], bf16, kind="ExternalInput").ap(),
        "anc": nc.dram_tensor("anc", [BS, D], bf16, kind="ExternalInput").ap(),
        "pos": nc.dram_tensor("pos", [BS, D], bf16, kind="ExternalInput").ap(),
        "neg": nc.dram_tensor("neg", [BS, D], bf16, kind="ExternalInput").ap(),
        "exem": nc.dram_tensor("exem", [C, D], bf16, kind="ExternalInput").ap(),
        "idxa": nc.dram_tensor("idxa", [128, 128], i16, kind="ExternalInput").ap(),
        "idxn": nc.dram_tensor("idxn", [128, 128], i16, kind="ExternalInput").ap(),
        "labsh": nc.dram_tensor("labsh", [P, NR], f32, kind="ExternalInput").ap(),
    }
    outs = {
        "partials": nc.dram_tensor("partials", [P, 4], f32, kind="ExternalOutput").ap()
    }
    with tile.TileContext(nc) as tc:
        _emit(tc, outs, ins, bases, W)
    nc.compile()
    _COMPILED[key] = nc
    return nc


def _wrap_idx(v):
    # dma_gather index layout: idx i at [i % 16, i // 16], replicated to
    # each 16-partition group (one per Q7 core).
    w = np.asarray(v, np.int16).reshape(128, 16).T  # [16, 128]
    return np.ascontiguousarray(np.tile(w, (8, 1)))  # [128, 128]


def _bf16(a):
    return np.ascontiguousarray(np.asarray(a, np.float32).astype(ml_dtypes.bfloat16))


def _prep(anchor, positive, negative, outputs, labels_anchor, labels_neg, exemplars):
    anchor = np.asarray(anchor, np.float32)
    positive = np.asarray(positive, np.float32)
    negative = np.asarray(negative, np.float32)
    outputs = np.asarray(outputs, np.float32)
    ex16 = _bf16(exemplars)
    la_all = np.asarray(labels_anchor).astype(np.int64)
    ln_all = np.asarray(labels_neg).astype(np.int64)

    cores = []
    lo = np.full(NB, C, np.int64)
    hi = np.full(NB, -1, np.int64)
    for k in range(NCORES):
        sl = slice(k * BS, (k + 1) * BS)
        la, ln = la_all[sl], ln_all[sl]
        pa = np.argsort(la, kind="stable")
        pn = np.argsort(ln, kind="stable")
        la_s, ln_s = la[pa], ln[pn]
        for v in (la_s, ln_s):
            vb = v.reshape(NB, P)
            np.minimum(lo, vb.min(axis=1), out=lo)
            np.maximum(hi, vb.max(axis=1), out=hi)
        cores.append((k, sl, pa, pn, la_s, ln_s))

    span = int((hi - lo).max()) + 1
    W = max(128, -(-span // 32) * 32)
    assert W <= C, f"label window infeasible: span {span}"
    bases = np.minimum(np.minimum(lo, C - W), hi - W + 1)
    bases = np.maximum(bases, 0).astype(np.int64)
    assert ((bases <= lo) & (bases + W > hi)).all()

    maps = []
    for k, sl, pa, pn, la_s, ln_s in cores:
        ln_pa = ln_all[sl][pa]
        xo = np.concatenate(
            [
                outputs[k * BS : (k + 1) * BS][pa],
                outputs[B + k * BS : B + (k + 1) * BS][pa],
                outputs[2 * B + k * BS : 2 * B + (k + 1) * BS][pn],
            ],
            axis=0,
        )
        labsh = np.empty((P, NR), np.float32)
        for t, v in enumerate((la_s, la_s, ln_s)):
            labsh[:, 16 * t : 16 * t + 16] = (
                (v.reshape(NB, P) - bases[:, None]).T.astype(np.float32)
            )
        maps.append(
            {
                "xout": _bf16(xo),
                "anc": _bf16(anchor[sl][pa]),
                "pos": _bf16(positive[sl][pa]),
                "neg": _bf16(negative[sl][pa]),
                "exem": ex16,
                "idxa": _wrap_idx(la_s),
                "idxn": _wrap_idx(ln_pa),
                "labsh": np.ascontiguousarray(labsh),
            }
        )
    return maps, tuple(int(b) for b in bases), W


def _combine(results):
    S = np.zeros(4, dtype=np.float64)
    for r in results:
        S += r["partials"].astype(np.float64).sum(axis=0)
    loss_softmax = (S[0] - S[1]) / (3 * B)
    loss_center = S[2]
    loss_triplet = S[3]
    loss_total = loss_softmax + 0.01 * loss_center + LAMBDA * loss_triplet
    return (
        np.float32(loss_total),
        np.float32(loss_triplet),
        np.float32(loss_softmax),
        np.float32(loss_center),
    )


def kernel(anchor, positive, negative, outputs, labels_anchor, labels_neg, exemplars):
    global LAST_RESULTS
    maps, bases, W = _prep(
        anchor, positive, negative, outputs, labels_anchor, labels_neg, exemplars
    )
    nc = _build(bases, W)
    res = run_bass_kernel_spmd(nc, maps, core_ids=list(range(NCORES)))
    LAST_RESULTS = res
    return _combine(res.results)


# revision 14
# speedup vs baseline: 1.4070x; 1.1373x over previous
"""Trainium2 Bass kernel for nn_ExemplarSoftmaxLoss (data-parallel over 8 cores).

v4 strategy:
  - Host-side: rows of each core's shard are PERMUTED (all reductions are
    permutation-invariant): distance rows + xout thirds 0/1 sorted by
    labels_anchor, xout third 2 sorted by labels_neg.  Sorted rows make
    each 128-row block's labels fall in a narrow window, so the label-logit
    extraction only scans a static W-column window.  Window bases/width are
    computed from the data before compile (kernel builds lazily).
  - All bulk inputs are uploaded as bf16 (the 2e-2 rel-err budget makes
    mixed precision the right kernel design): halves HBM traffic to
    ~22.8 MB/core (~64 us of DMA) and enables the DVE 2x bf16 mode for
    the distance diffs.
  - The distance phase runs in a TRANSPOSED layout: anchor/pos/neg are
    uploaded as [D, BS] and exemplar rows are fetched with
    dma_gather(transpose=True), so diff tiles are [d-partition, row-free].
    Row sum-of-squares then runs on the otherwise-idle TensorEngine as
    diagonal matmuls df.T @ df (PSUM-accumulated over the 4 d-chunks);
    the diagonal is pulled out with a 128-wide is_equal STT.  This removes
    all 96 square ops (~60 us of Scalar+DVE) from the critical engines.
  - ScalarE runs the exp stream only; DVE does extraction + diffs + diag.
  - Host: float64 reduction of the 8x[128,4] partials -> 4 scalar losses.
"""

import os
import sys

import numpy as np
import ml_dtypes

for _p in ("/opt/trn_rl_repo",):
    if _p not in sys.path and os.path.isdir(_p):
        sys.path.insert(0, _p)

import concourse.bass as bass
import concourse.tile as tile
from concourse import bacc, mybir
from concourse._compat import with_exitstack
from concourse.bass_utils import run_bass_kernel_spmd

try:
    import antenv.axon_hooks  # noqa: F401
except ImportError:
    import types as _types

    _m = _types.ModuleType("antenv.axon_hooks")
    _m.get_axon_ntff_profile_hook = lambda: None
    _m.set_axon_ntff_profile_hook = lambda h: None
    sys.modules["antenv.axon_hooks"] = _m

# Problem constants (hardcoded per the harness contract).
B, D, C = 16384, 512, 1000
NCORES = 8
BS = B // NCORES  # 2048 batch rows per core
RS = 3 * BS  # 6144 softmax rows per core
P = 128
NB = BS // P  # 16 row-blocks in the distance phase
NR = RS // P  # 48 row-blocks in the softmax phase
NG = 4  # groups of 4 row-blocks in the distance phase
DC = D // P  # 4 d-chunks in the transposed layout
EPS = 1e-6
MARGIN2 = 0.2
LAMBDA = 1.0

# xout DMA tiles: n blocks each; 2-block head tiles give the exp stream an
# early start; the rest are 4-block (1MB bf16) tiles.
TILE_SHAPES = [2, 2, 4, 4, 4]
TILES = [
    (t, sum(TILE_SHAPES[:i]), TILE_SHAPES[i])
    for t in range(3)
    for i in range(len(TILE_SHAPES))
]
NXT = len(TILES)  # 15

f32 = mybir.dt.float32
bf16 = mybir.dt.bfloat16
i16 = mybir.dt.int16
Alu = mybir.AluOpType
Act = mybir.ActivationFunctionType
AX = mybir.AxisListType

LAST_RESULTS = None  # BassKernelResults of the most recent run (for test.py)


@with_exitstack
def _emit(ctx, tc, outs, ins, bases, W):
    nc = tc.nc
    xo = ins["xout"]  # [RS, C] bf16 (3 thirds, host-permuted)
    aa = ins["anc"]  # [D, BS] bf16 transposed (cols sorted by la)
    pp = ins["pos"]  # [D, BS] bf16
    ng = ins["neg"]  # [D, BS] bf16
    ex = ins["exem"]  # [C, D]  bf16 exemplar table
    ia = ins["idxa"]  # [128, 128] i16 wrapped gather idx (= sorted la)
    in_ = ins["idxn"]  # [128, 128] i16 wrapped gather idx (= ln[perm_a])
    lsh = ins["labsh"]  # [P, NR] f32 label - window_base per block
    pd = outs["partials"]  # [P, 4] f32

    sing = ctx.enter_context(tc.tile_pool(name="sing", bufs=1))
    xpool = ctx.enter_context(tc.tile_pool(name="xp", bufs=3))
    ejp = ctx.enter_context(tc.tile_pool(name="ejp", bufs=2, space="PSUM"))
    mmp = ctx.enter_context(tc.tile_pool(name="mmp", bufs=4, space="PSUM"))
    ljp = ctx.enter_context(tc.tile_pool(name="ljp", bufs=2))
    dgp = ctx.enter_context(tc.tile_pool(name="dgp", bufs=3))
    dfp = ctx.enter_context(tc.tile_pool(name="dfp", bufs=3))

    sums = sing.tile([P, NR], f32)  # per-row sum(exp(x))
    lbl = sing.tile([P, NR], f32)  # label logits per block
    d2a = sing.tile([P, NB * 3], f32)  # sq dists: dr1,dn1,dr2
    d2v = sing.tile([P, NB * 3], f32)  # sq dists: dn2,tp,tn
    iota_w = sing.tile([P, W], f32)
    pidx = sing.tile([P, 1], f32)  # value = partition index
    lsh_t = sing.tile([P, NR], f32)
    ia_t = sing.tile([128, 128], i16)
    in_t = sing.tile([128, 128], i16)
    # transposed distance operands: tile[p, c, r] = X[r, c*128+p]
    at = sing.tile([P, DC, BS], bf16)
    pt = sing.tile([P, DC, BS], bf16)
    nt = sing.tile([P, DC, BS], bf16)
    # gather output must have contiguous free dims per call -> group-major
    exa = sing.tile([P, NG, DC, 512], bf16)
    exn = sing.tile([P, NG, DC, 512], bf16)

    # small loads + ALL gathers lead the SWDGE queue (Q7 gather emission is
    # ~4.7us per 512 rows; starting at ~11us they finish by ~50us)
    nc.gpsimd.dma_start(out=lsh_t[:], in_=lsh[:])
    nc.gpsimd.dma_start(out=ia_t[:], in_=ia[:])
    nc.gpsimd.dma_start(out=in_t[:], in_=in_[:])
    nc.gpsimd.iota(
        iota_w[:],
        pattern=[[1, W]],
        base=0,
        channel_multiplier=0,
        allow_small_or_imprecise_dtypes=True,
    )
    nc.gpsimd.iota(
        pidx[:],
        pattern=[[1, 1]],
        base=0,
        channel_multiplier=1,
        allow_small_or_imprecise_dtypes=True,
    )
    for g in range(NG):
        for dst, idx in ((exa, ia_t), (exn, in_t)):
            nc.gpsimd.dma_gather(
                dst[:, g],
                ex[:],
                idx[:, 32 * g : 32 * g + 32],
                512,
                512,
                D,
                transpose=True,
            )

    xo3 = xo.rearrange("(t r) c -> t r c", t=3)
    xt_tiles = {}

    def emit_xload(s):
        t, j0, nb = TILES[s]
        xt = xpool.tile([P, nb, C], bf16, tag="xt", name=f"xt{s}")
        nc.sync.dma_start(
            out=xt[:],
            in_=xo3[t, j0 * P : (j0 + nb) * P, :].rearrange(
                "(t p) c -> p t c", p=P
            ),
        )
        xt_tiles[s] = xt

    def emit_xcompute(s):
        xt = xt_tiles.pop(s)
        t, j0, nb = TILES[s]
        for b in range(nb):
            j = j0 + b  # block index within the third
            col = 16 * t + j
            ej = ejp.tile([P, C], f32, tag="ej")
            nc.scalar.activation(
                out=ej[:],
                in_=xt[:, b, :],
                func=Act.Exp,
                accum_out=sums[:, col : col + 1],
            )
            base = bases[j]
            lj = ljp.tile([P, W], f32, tag="lj")
            nc.vector.scalar_tensor_tensor(
                out=lj[:],
                in0=iota_w[:],
                scalar=lsh_t[:, col : col + 1],
                in1=xt[:, b, base : base + W],
                op0=Alu.is_equal,
                op1=Alu.mult,
                accum_out=lbl[:, col : col + 1],
            )

    def emit_apn_loads(g):
        r0, r1 = 512 * g, 512 * (g + 1)
        for dst, src in ((at, aa), (pt, pp), (nt, ng)):
            nc.sync.dma_start(
                out=dst[:, :, r0:r1],
                in_=src[:, r0:r1].rearrange("(c p) r -> p c r", p=P),
            )

    def emit_diag(df, rcl, d2t, ci, g):
        # mm = df_chunk.T @ df_chunk accumulated over the 4 d-chunks;
        # diag(mm)[p] = sum_d df[d, blk*128+p]^2 = d^2 of row blk*128+p
        blk = 4 * g + rcl
        rsl = slice(128 * rcl, 128 * (rcl + 1))
        mm = mmp.tile([P, P], f32, tag="mm")
        for dc in range(DC):
            nc.tensor.matmul(
                out=mm[:],
                lhsT=df[:, dc, rsl],
                rhs=df[:, dc, rsl],
                start=(dc == 0),
                stop=(dc == DC - 1),
            )
        dg = dgp.tile([P, P], f32, tag="dg")
        nc.vector.scalar_tensor_tensor(
            out=dg[:],
            in0=iota_w[:, 0:P],
            scalar=pidx[:],
            in1=mm[:],
            op0=Alu.is_equal,
            op1=Alu.mult,
            accum_out=d2t[:, blk * 3 + ci : blk * 3 + ci + 1],
        )

    def emit_group(g):
        rsl = slice(512 * g, 512 * (g + 1))
        pairs = (
            (at, exa, d2a, 0),  # d_ref1
            (nt, exa, d2a, 1),  # d_neg1
            (at, exn, d2a, 2),  # d_ref2
            (nt, exn, d2v, 0),  # d_neg2
            (at, pt, d2v, 1),  # tp
            (at, nt, d2v, 2),  # tn
        )
        dfs = []
        # software-pipeline: diffs lead their diag extraction by one pair so
        # the DVE never waits on the PE matmuls
        for pi, (xs, ys, d2t, ci) in enumerate(pairs):
            in0 = xs[:, :, rsl]
            in1 = ys[:, g] if (ys is exa or ys is exn) else ys[:, :, rsl]
            df = dfp.tile([P, DC, 512], bf16, tag="df")
            nc.vector.tensor_tensor(
                out=df[:], in0=in0, in1=in1, op=Alu.subtract
            )
            dfs.append((df, d2t, ci))
            if pi >= 1:
                df0, d2t0, ci0 = dfs[pi - 1]
                for rcl in range(4):
                    emit_diag(df0, rcl, d2t0, ci0, g)
        df0, d2t0, ci0 = dfs[-1]
        for rcl in range(4):
            emit_diag(df0, rcl, d2t0, ci0, g)

    # ---- main schedule ----
    emit_xload(0)
    emit_xload(1)
    emit_apn_loads(0)
    emit_xload(2)
    emit_apn_loads(1)

    for s in range(NXT):
        if s + 3 < NXT:
            emit_xload(s + 3)
        if s == 2:
            emit_apn_loads(2)
        if s == 4:
            emit_apn_loads(3)
        emit_xcompute(s)
        if s in (4, 6, 8, 10):
            emit_group((s - 4) // 2)

    # ---- tail ----
    part = sing.tile([P, 4], f32)
    dda = sing.tile([P, NB * 3], f32)
    ddv = sing.tile([P, NB * 3], f32)
    nc.scalar.activation(out=dda[:], in_=d2a[:], func=Act.Sqrt)
    nc.scalar.activation(out=ddv[:], in_=d2v[:], func=Act.Sqrt)
    logs = sing.tile([P, NR], f32)
    nc.scalar.activation(out=logs[:], in_=sums[:], func=Act.Ln)
    nc.vector.reduce_sum(out=part[:, 0:1], in_=logs[:], axis=AX.X)
    nc.vector.reduce_sum(out=part[:, 1:2], in_=lbl[:], axis=AX.X)

    dA = dda[:].rearrange("p (b k) -> p b k", k=3)
    dV = ddv[:].rearrange("p (b k) -> p b k", k=3)

    x1 = sing.tile([P, NB], f32)
    m1 = sing.tile([P, NB], f32)
    c1 = sing.tile([P, NB], f32)
    x2 = sing.tile([P, NB], f32)
    c2 = sing.tile([P, NB], f32)
    x3 = sing.tile([P, NB], f32)
    t3 = sing.tile([P, NB], f32)
    ca = sing.tile([P, 1], f32)
    cb = sing.tile([P, 1], f32)

    # c1 = (dr1 - dn1 > 0) ? (dr1 - dn1 + MARGIN2) : 0
    nc.vector.tensor_tensor(out=x1[:], in0=dA[:, :, 0], in1=dA[:, :, 1], op=Alu.subtract)
    nc.vector.tensor_scalar(
        out=m1[:], in0=x1[:], scalar1=0.0, scalar2=None, op0=Alu.is_gt
    )
    nc.vector.scalar_tensor_tensor(
        out=c1[:], in0=x1[:], scalar=MARGIN2, in1=m1[:],
        op0=Alu.add, op1=Alu.mult, accum_out=ca[:],
    )
    # c2 = relu(dn2 - dr2)
    nc.vector.tensor_tensor(out=x2[:], in0=dV[:, :, 0], in1=dA[:, :, 2], op=Alu.subtract)
    nc.vector.tensor_scalar(
        out=c2[:], in0=x2[:], scalar1=0.0, scalar2=None,
        op0=Alu.max, op1=Alu.add, accum_out=cb[:],
    )
    # t = relu(tp - tn)
    nc.vector.tensor_tensor(out=x3[:], in0=dV[:, :, 1], in1=dV[:, :, 2], op=Alu.subtract)
    nc.vector.tensor_scalar(
        out=t3[:], in0=x3[:], scalar1=0.0, scalar2=None,
        op0=Alu.max, op1=Alu.add, accum_out=part[:, 3:4],
    )
    nc.vector.tensor_tensor(out=part[:, 2:3], in0=ca[:], in1=cb[:], op=Alu.add)
    nc.sync.dma_start(out=pd[:], in_=part[:])


_COMPILED = {}


def _build(bases, W):
    key = (tuple(bases), W)
    if key in _COMPILED:
        return _COMPILED[key]
    nc = bacc.Bacc(
        "TRN2",
        target_bir_lowering=False,
        debug=False,
        enable_asserts=False,
        num_devices=NCORES,
    )
    ins = {
        "xout": nc.dram_tensor("xout", [RS, C], bf16, kind="ExternalInput").ap(),
        "anc": nc.dram_tensor("anc", [D, BS], bf16, kind="ExternalInput").ap(),
        "pos": nc.dram_tensor("pos", [D, BS], bf16, kind="ExternalInput").ap(),
        "neg": nc.dram_tensor("neg", [D, BS], bf16, kind="ExternalInput").ap(),
        "exem": nc.dram_tensor("exem", [C, D], bf16, kind="ExternalInput").ap(),
        "idxa": nc.dram_tensor("idxa", [128, 128], i16, kind="ExternalInput").ap(),
        "idxn": nc.dram_tensor("idxn", [128, 128], i16, kind="ExternalInput").ap(),
        "labsh": nc.dram_tensor("labsh", [P, NR], f32, kind="ExternalInput").ap(),
    }
    outs = {
        "partials": nc.dram_tensor("partials", [P, 4], f32, kind="ExternalOutput").ap()
    }
    with tile.TileContext(nc) as tc:
        _emit(tc, outs, ins, bases, W)
    nc.compile()
    _COMPILED[key] = nc
    return nc


def _wrap_idx(v):
    # dma_gather index layout: idx i at [i % 16, i // 16], replicated to
    # each 16-partition group (one per Q7 core).
    w = np.asarray(v, np.int16).reshape(128, 16).T  # [16, 128]
    return np.ascontiguousarray(np.tile(w, (8, 1)))  # [128, 128]


def _bf16(a):
    return np.ascontiguousarray(np.asarray(a, np.float32).astype(ml_dtypes.bfloat16))


def _prep(anchor, positive, negative, outputs, labels_anchor, labels_neg, exemplars):
    anchor = np.asarray(anchor, np.float32)
    positive = np.asarray(positive, np.float32)
    negative = np.asarray(negative, np.float32)
    outputs = np.asarray(outputs, np.float32)
    ex16 = _bf16(exemplars)
    la_all = np.asarray(labels_anchor).astype(np.int64)
    ln_all = np.asarray(labels_neg).astype(np.int64)

    cores = []
    lo = np.full(NB, C, np.int64)
    hi = np.full(NB, -1, np.int64)
    for k in range(NCORES):
        sl = slice(k * BS, (k + 1) * BS)
        la, ln = la_all[sl], ln_all[sl]
        pa = np.argsort(la, kind="stable")
        pn = np.argsort(ln, kind="stable")
        la_s, ln_s = la[pa], ln[pn]
        for v in (la_s, ln_s):
            vb = v.reshape(NB, P)
            np.minimum(lo, vb.min(axis=1), out=lo)
            np.maximum(hi, vb.max(axis=1), out=hi)
        cores.append((k, sl, pa, pn, la_s, ln_s))

    span = int((hi - lo).max()) + 1
    W = max(128, -(-span // 32) * 32)
    assert W <= C, f"label window infeasible: span {span}"
    bases = np.minimum(np.minimum(lo, C - W), hi - W + 1)
    bases = np.maximum(bases, 0).astype(np.int64)
    assert ((bases <= lo) & (bases + W > hi)).all()

    maps = []
    for k, sl, pa, pn, la_s, ln_s in cores:
        ln_pa = ln_all[sl][pa]
        xo = np.concatenate(
            [
                outputs[k * BS : (k + 1) * BS][pa],
                outputs[B + k * BS : B + (k + 1) * BS][pa],
                outputs[2 * B + k * BS : 2 * B + (k + 1) * BS][pn],
            ],
            axis=0,
        )
        labsh = np.empty((P, NR), np.float32)
        for t, v in enumerate((la_s, la_s, ln_s)):
            labsh[:, 16 * t : 16 * t + 16] = (
                (v.reshape(NB, P) - bases[:, None]).T.astype(np.float32)
            )
        maps.append(
            {
                "xout": _bf16(xo),
                "anc": _bf16(anchor[sl][pa].T),
                "pos": _bf16(positive[sl][pa].T),
                "neg": _bf16(negative[sl][pa].T),
                "exem": ex16,
                "idxa": _wrap_idx(la_s),
                "idxn": _wrap_idx(ln_pa),
                "labsh": np.ascontiguousarray(labsh),
            }
        )
    return maps, tuple(int(b) for b in bases), W


def _combine(results):
    S = np.zeros(4, dtype=np.float64)
    for r in results:
        S += r["partials"].astype(np.float64).sum(axis=0)
    loss_softmax = (S[0] - S[1]) / (3 * B)
    loss_center = S[2]
    loss_triplet = S[3]
    loss_total = loss_softmax + 0.01 * loss_center + LAMBDA * loss_triplet
    return (
        np.float32(loss_total),
        np.float32(loss_triplet),
        np.float32(loss_softmax),
        np.float32(loss_center),
    )


def kernel(anchor, positive, negative, outputs, labels_anchor, labels_neg, exemplars):
    global LAST_RESULTS
    maps, bases, W = _prep(
        anchor, positive, negative, outputs, labels_anchor, labels_neg, exemplars
    )
    nc = _build(bases, W)
    res = run_bass_kernel_spmd(nc, maps, core_ids=list(range(NCORES)))
    LAST_RESULTS = res
    return _combine(res.results)


# revision 18
# speedup vs baseline: 1.5810x; 1.1237x over previous
"""Trainium2 Bass kernel for nn_ExemplarSoftmaxLoss (data-parallel over 8 cores).

v4 strategy:
  - Host-side: rows of each core's shard are PERMUTED (all reductions are
    permutation-invariant): distance rows + xout thirds 0/1 sorted by
    labels_anchor, xout third 2 sorted by labels_neg.  Sorted rows make
    each 128-row block's labels fall in a narrow window, so the label-logit
    extraction only scans a static W-column window.  Window bases/width are
    computed from the data before compile (kernel builds lazily).
  - All bulk inputs are uploaded as bf16 (the 2e-2 rel-err budget makes
    mixed precision the right kernel design): halves HBM traffic to
    ~22.8 MB/core (~64 us of DMA) and enables the DVE 2x bf16 mode for
    the distance diffs.
  - The distance phase runs in a TRANSPOSED layout: anchor/pos/neg are
    uploaded as [D, BS] and exemplar rows are fetched with
    dma_gather(transpose=True), so diff tiles are [d-partition, row-free].
    Row sum-of-squares then runs on the otherwise-idle TensorEngine as
    diagonal matmuls df.T @ df (PSUM-accumulated over the 4 d-chunks);
    the diagonal is pulled out with a 128-wide is_equal STT.  This removes
    all 96 square ops (~60 us of Scalar+DVE) from the critical engines.
  - ScalarE runs the exp stream only; DVE does extraction + diffs + diag.
  - Host: float64 reduction of the 8x[128,4] partials -> 4 scalar losses.
"""

import os
import sys

import numpy as np
import ml_dtypes

for _p in ("/opt/trn_rl_repo",):
    if _p not in sys.path and os.path.isdir(_p):
        sys.path.insert(0, _p)

import concourse.bass as bass
import concourse.tile as tile
from concourse import bacc, mybir
from concourse._compat import with_exitstack
from concourse.bass_utils import run_bass_kernel_spmd

try:
    import antenv.axon_hooks  # noqa: F401
except ImportError:
    import types as _types

    _m = _types.ModuleType("antenv.axon_hooks")
    _m.get_axon_ntff_profile_hook = lambda: None
    _m.set_axon_ntff_profile_hook = lambda h: None
    sys.modules["antenv.axon_hooks"] = _m

# Problem constants (hardcoded per the harness contract).
B, D, C = 16384, 512, 1000
NCORES = 8
BS = B // NCORES  # 2048 batch rows per core
RS = 3 * BS  # 6144 softmax rows per core
P = 128
NB = BS // P  # 16 row-blocks in the distance phase
NR = RS // P  # 48 row-blocks in the softmax phase
NG = 4  # groups of 4 row-blocks in the distance phase
DC = D // P  # 4 d-chunks in the transposed layout
EPS = 1e-6
MARGIN2 = 0.2
LAMBDA = 1.0

# xout DMA tiles: n blocks each; 2-block head tiles give the exp stream an
# early start; the rest are 4-block (1MB bf16) tiles.
TILE_SHAPES = [2, 2, 4, 4, 4]
TILES = [
    (t, sum(TILE_SHAPES[:i]), TILE_SHAPES[i])
    for t in range(3)
    for i in range(len(TILE_SHAPES))
]
NXT = len(TILES)  # 15

f32 = mybir.dt.float32
bf16 = mybir.dt.bfloat16
i16 = mybir.dt.int16
Alu = mybir.AluOpType
Act = mybir.ActivationFunctionType
AX = mybir.AxisListType

LAST_RESULTS = None  # BassKernelResults of the most recent run (for test.py)


@with_exitstack
def _emit(ctx, tc, outs, ins, bases, W):
    nc = tc.nc
    xo = ins["xout"]  # [RS, C] bf16 (3 thirds, host-permuted)
    aa = ins["anc"]  # [D, BS] bf16 transposed (cols sorted by la)
    pp = ins["pos"]  # [D, BS] bf16
    ng = ins["neg"]  # [D, BS] bf16
    ex = ins["exem"]  # [C, D]  bf16 exemplar table
    ia = ins["idxa"]  # [128, 128] i16 wrapped gather idx (= sorted la)
    in_ = ins["idxn"]  # [128, 128] i16 wrapped gather idx (= ln[perm_a])
    lsh = ins["labsh"]  # [P, NR] f32 label - window_base per block
    pd = outs["partials"]  # [P, 4] f32

    sing = ctx.enter_context(tc.tile_pool(name="sing", bufs=1))
    xpool = ctx.enter_context(tc.tile_pool(name="xp", bufs=6))
    ejp = ctx.enter_context(tc.tile_pool(name="ejp", bufs=2, space="PSUM"))
    mmp = ctx.enter_context(tc.tile_pool(name="mmp", bufs=4, space="PSUM"))
    ljp = ctx.enter_context(tc.tile_pool(name="ljp", bufs=3))
    dgp = ctx.enter_context(tc.tile_pool(name="dgp", bufs=4))
    dfp = ctx.enter_context(tc.tile_pool(name="dfp", bufs=4))

    sums = sing.tile([P, NR], f32)  # per-row sum(exp(x))
    lbl = sing.tile([P, NR], f32)  # label logits per block
    d2a = sing.tile([P, NB * 3], f32)  # sq dists: dr1,dn1,dr2
    d2v = sing.tile([P, NB * 3], f32)  # sq dists: dn2,tp,tn
    iota_w = sing.tile([P, W], f32)
    pidx = sing.tile([P, 1], f32)  # value = partition index
    lsh_t = sing.tile([P, NR], f32)
    ia_t = sing.tile([128, 128], i16)
    in_t = sing.tile([128, 128], i16)
    # transposed distance operands: tile[p, c, r] = X[r, c*128+p]
    at = sing.tile([P, DC, BS], bf16)
    pt = sing.tile([P, DC, BS], bf16)
    nt = sing.tile([P, DC, BS], bf16)
    # gather output must have contiguous free dims per call -> group-major
    exa = sing.tile([P, NG, DC, 512], bf16)
    exn = sing.tile([P, NG, DC, 512], bf16)

    # small loads go on the sync HWDGE queue: the pool-dynamic SDMA queue is
    # starved while the sync queue streams, which would delay the first
    # gather (which waits on the idx loads) by ~16us.
    nc.sync.dma_start(out=lsh_t[:], in_=lsh[:])
    nc.sync.dma_start(out=ia_t[:], in_=ia[:])
    nc.sync.dma_start(out=in_t[:], in_=in_[:])
    nc.gpsimd.iota(
        iota_w[:],
        pattern=[[1, W]],
        base=0,
        channel_multiplier=0,
        allow_small_or_imprecise_dtypes=True,
    )
    nc.gpsimd.iota(
        pidx[:],
        pattern=[[1, 1]],
        base=0,
        channel_multiplier=1,
        allow_small_or_imprecise_dtypes=True,
    )
    for g in range(NG):
        for dst, idx in ((exa, ia_t), (exn, in_t)):
            nc.gpsimd.dma_gather(
                dst[:, g],
                ex[:],
                idx[:, 32 * g : 32 * g + 32],
                512,
                512,
                D,
                transpose=True,
            )

    xo3 = xo.rearrange("(t r) c -> t r c", t=3)
    xt_tiles = {}

    def emit_xload(s):
        t, j0, nb = TILES[s]
        xt = xpool.tile([P, nb, C], bf16, tag="xt", name=f"xt{s}")
        nc.sync.dma_start(
            out=xt[:],
            in_=xo3[t, j0 * P : (j0 + nb) * P, :].rearrange(
                "(t p) c -> p t c", p=P
            ),
        )
        xt_tiles[s] = xt

    def emit_xcompute(s):
        xt = xt_tiles.pop(s)
        t, j0, nb = TILES[s]
        for b in range(nb):
            j = j0 + b  # block index within the third
            col = 16 * t + j
            ej = ejp.tile([P, C], f32, tag="ej")
            nc.scalar.activation(
                out=ej[:],
                in_=xt[:, b, :],
                func=Act.Exp,
                accum_out=sums[:, col : col + 1],
            )
            base = bases[j]
            lj = ljp.tile([P, W], f32, tag="lj")
            nc.vector.scalar_tensor_tensor(
                out=lj[:],
                in0=iota_w[:],
                scalar=lsh_t[:, col : col + 1],
                in1=xt[:, b, base : base + W],
                op0=Alu.is_equal,
                op1=Alu.mult,
                accum_out=lbl[:, col : col + 1],
            )

    def emit_apn_loads(g):
        # scalar-engine HWDGE ring: separate FIFO from the sync queue, so
        # these don't delay the xout tile stream
        r0, r1 = 512 * g, 512 * (g + 1)
        for dst, src in ((at, aa), (pt, pp), (nt, ng)):
            nc.scalar.dma_start(
                out=dst[:, :, r0:r1],
                in_=src[:, r0:r1].rearrange("(c p) r -> p c r", p=P),
            )

    def emit_diag(df, rcl, d2t, ci, g):
        # mm = df_chunk.T @ df_chunk accumulated over the 4 d-chunks;
        # diag(mm)[p] = sum_d df[d, blk*128+p]^2 = d^2 of row blk*128+p
        blk = 4 * g + rcl
        rsl = slice(128 * rcl, 128 * (rcl + 1))
        mm = mmp.tile([P, P], f32, tag="mm")
        for dc in range(DC):
            nc.tensor.matmul(
                out=mm[:],
                lhsT=df[:, dc, rsl],
                rhs=df[:, dc, rsl],
                start=(dc == 0),
                stop=(dc == DC - 1),
            )
        dg = dgp.tile([P, P], f32, tag="dg")
        nc.vector.scalar_tensor_tensor(
            out=dg[:],
            in0=iota_w[:, 0:P],
            scalar=pidx[:],
            in1=mm[:],
            op0=Alu.is_equal,
            op1=Alu.mult,
            accum_out=d2t[:, blk * 3 + ci : blk * 3 + ci + 1],
        )

    def emit_group(g):
        rsl = slice(512 * g, 512 * (g + 1))
        pairs = (
            (at, exa, d2a, 0),  # d_ref1
            (nt, exa, d2a, 1),  # d_neg1
            (at, exn, d2a, 2),  # d_ref2
            (nt, exn, d2v, 0),  # d_neg2
            (at, pt, d2v, 1),  # tp
            (at, nt, d2v, 2),  # tn
        )
        dfs = []
        # software-pipeline: diffs lead their diag extraction by one pair so
        # the DVE never waits on the PE matmuls
        for pi, (xs, ys, d2t, ci) in enumerate(pairs):
            in0 = xs[:, :, rsl]
            in1 = ys[:, g] if (ys is exa or ys is exn) else ys[:, :, rsl]
            df = dfp.tile([P, DC, 512], bf16, tag="df")
            nc.vector.tensor_tensor(
                out=df[:], in0=in0, in1=in1, op=Alu.subtract
            )
            dfs.append((df, d2t, ci))
            if pi >= 1:
                df0, d2t0, ci0 = dfs[pi - 1]
                for rcl in range(4):
                    emit_diag(df0, rcl, d2t0, ci0, g)
        df0, d2t0, ci0 = dfs[-1]
        for rcl in range(4):
            emit_diag(df0, rcl, d2t0, ci0, g)

    # ---- main schedule ----
    emit_xload(0)
    emit_xload(1)
    emit_apn_loads(0)
    emit_xload(2)
    emit_apn_loads(1)

    for s in range(NXT):
        if s + 3 < NXT:
            emit_xload(s + 3)
        if s == 2:
            emit_apn_loads(2)
        if s == 4:
            emit_apn_loads(3)
        emit_xcompute(s)
        if s in (4, 6, 8, 10):
            emit_group((s - 4) // 2)

    # ---- tail ----
    part = sing.tile([P, 4], f32)
    dda = sing.tile([P, NB * 3], f32)
    ddv = sing.tile([P, NB * 3], f32)
    nc.scalar.activation(out=dda[:], in_=d2a[:], func=Act.Sqrt)
    nc.scalar.activation(out=ddv[:], in_=d2v[:], func=Act.Sqrt)
    logs = sing.tile([P, NR], f32)
    nc.scalar.activation(out=logs[:], in_=sums[:], func=Act.Ln)
    nc.vector.reduce_sum(out=part[:, 0:1], in_=logs[:], axis=AX.X)
    nc.vector.reduce_sum(out=part[:, 1:2], in_=lbl[:], axis=AX.X)

    dA = dda[:].rearrange("p (b k) -> p b k", k=3)
    dV = ddv[:].rearrange("p (b k) -> p b k", k=3)

    x1 = sing.tile([P, NB], f32)
    m1 = sing.tile([P, NB], f32)
    c1 = sing.tile([P, NB], f32)
    x2 = sing.tile([P, NB], f32)
    c2 = sing.tile([P, NB], f32)
    x3 = sing.tile([P, NB], f32)
    t3 = sing.tile([P, NB], f32)
    ca = sing.tile([P, 1], f32)
    cb = sing.tile([P, 1], f32)

    # c1 = (dr1 - dn1 > 0) ? (dr1 - dn1 + MARGIN2) : 0
    nc.vector.tensor_tensor(out=x1[:], in0=dA[:, :, 0], in1=dA[:, :, 1], op=Alu.subtract)
    nc.vector.tensor_scalar(
        out=m1[:], in0=x1[:], scalar1=0.0, scalar2=None, op0=Alu.is_gt
    )
    nc.vector.scalar_tensor_tensor(
        out=c1[:], in0=x1[:], scalar=MARGIN2, in1=m1[:],
        op0=Alu.add, op1=Alu.mult, accum_out=ca[:],
    )
    # c2 = relu(dn2 - dr2)
    nc.vector.tensor_tensor(out=x2[:], in0=dV[:, :, 0], in1=dA[:, :, 2], op=Alu.subtract)
    nc.vector.tensor_scalar(
        out=c2[:], in0=x2[:], scalar1=0.0, scalar2=None,
        op0=Alu.max, op1=Alu.add, accum_out=cb[:],
    )
    # t = relu(tp - tn)
    nc.vector.tensor_tensor(out=x3[:], in0=dV[:, :, 1], in1=dV[:, :, 2], op=Alu.subtract)
    nc.vector.tensor_scalar(
        out=t3[:], in0=x3[:], scalar1=0.0, scalar2=None,
        op0=Alu.max, op1=Alu.add, accum_out=part[:, 3:4],
    )
    nc.vector.tensor_tensor(out=part[:, 2:3], in0=ca[:], in1=cb[:], op=Alu.add)
    nc.sync.dma_start(out=pd[:], in_=part[:])


_COMPILED = {}


def _build(bases, W):
    key = (tuple(bases), W)
    if key in _COMPILED:
        return _COMPILED[key]
    nc = bacc.Bacc(
        "TRN2",
        target_bir_lowering=False,
        debug=False,
        enable_asserts=False,
        num_devices=NCORES,
    )
    ins = {
        "xout": nc.dram_tensor("xout", [RS, C], bf16, kind="ExternalInput").ap(),
        "anc": nc.dram_tensor("anc", [D, BS], bf16, kind="ExternalInput").ap(),
        "pos": nc.dram_tensor("pos", [D, BS], bf16, kind="ExternalInput").ap(),
        "neg": nc.dram_tensor("neg", [D, BS], bf16, kind="ExternalInput").ap(),
        "exem": nc.dram_tensor("exem", [C, D], bf16, kind="ExternalInput").ap(),
        "idxa": nc.dram_tensor("idxa", [128, 128], i16, kind="ExternalInput").ap(),
        "idxn": nc.dram_tensor("idxn", [128, 128], i16, kind="ExternalInput").ap(),
        "labsh": nc.dram_tensor("labsh", [P, NR], f32, kind="ExternalInput").ap(),
    }
    outs = {
        "partials": nc.dram_tensor("partials", [P, 4], f32, kind="ExternalOutput").ap()
    }
    with tile.TileContext(nc) as tc:
        _emit(tc, outs, ins, bases, W)
    nc.compile()
    _COMPILED[key] = nc
    return nc


def _wrap_idx(v):
    # dma_gather index layout: idx i at [i % 16, i // 16], replicated to
    # each 16-partition group (one per Q7 core).
    w = np.asarray(v, np.int16).reshape(128, 16).T  # [16, 128]
    return np.ascontiguousarray(np.tile(w, (8, 1)))  # [128, 128]


def _bf16(a):
    return np.ascontiguousarray(np.asarray(a, np.float32).astype(ml_dtypes.bfloat16))


def _prep(anchor, positive, negative, outputs, labels_anchor, labels_neg, exemplars):
    anchor = np.asarray(anchor, np.float32)
    positive = np.asarray(positive, np.float32)
    negative = np.asarray(negative, np.float32)
    outputs = np.asarray(outputs, np.float32)
    ex16 = _bf16(exemplars)
    la_all = np.asarray(labels_anchor).astype(np.int64)
    ln_all = np.asarray(labels_neg).astype(np.int64)

    cores = []
    lo = np.full(NB, C, np.int64)
    hi = np.full(NB, -1, np.int64)
    for k in range(NCORES):
        sl = slice(k * BS, (k + 1) * BS)
        la, ln = la_all[sl], ln_all[sl]
        pa = np.argsort(la, kind="stable")
        pn = np.argsort(ln, kind="stable")
        la_s, ln_s = la[pa], ln[pn]
        for v in (la_s, ln_s):
            vb = v.reshape(NB, P)
            np.minimum(lo, vb.min(axis=1), out=lo)
            np.maximum(hi, vb.max(axis=1), out=hi)
        cores.append((k, sl, pa, pn, la_s, ln_s))

    span = int((hi - lo).max()) + 1
    W = max(128, -(-span // 32) * 32)
    assert W <= C, f"label window infeasible: span {span}"
    bases = np.minimum(np.minimum(lo, C - W), hi - W + 1)
    bases = np.maximum(bases, 0).astype(np.int64)
    assert ((bases <= lo) & (bases + W > hi)).all()

    maps = []
    for k, sl, pa, pn, la_s, ln_s in cores:
        ln_pa = ln_all[sl][pa]
        xo = np.concatenate(
            [
                outputs[k * BS : (k + 1) * BS][pa],
                outputs[B + k * BS : B + (k + 1) * BS][pa],
                outputs[2 * B + k * BS : 2 * B + (k + 1) * BS][pn],
            ],
            axis=0,
        )
        labsh = np.empty((P, NR), np.float32)
        for t, v in enumerate((la_s, la_s, ln_s)):
            labsh[:, 16 * t : 16 * t + 16] = (
                (v.reshape(NB, P) - bases[:, None]).T.astype(np.float32)
            )
        maps.append(
            {
                "xout": _bf16(xo),
                "anc": _bf16(anchor[sl][pa].T),
                "pos": _bf16(positive[sl][pa].T),
                "neg": _bf16(negative[sl][pa].T),
                "exem": ex16,
                "idxa": _wrap_idx(la_s),
                "idxn": _wrap_idx(ln_pa),
                "labsh": np.ascontiguousarray(labsh),
            }
        )
    return maps, tuple(int(b) for b in bases), W


def _combine(results):
    S = np.zeros(4, dtype=np.float64)
    for r in results:
        S += r["partials"].astype(np.float64).sum(axis=0)
    loss_softmax = (S[0] - S[1]) / (3 * B)
    loss_center = S[2]
    loss_triplet = S[3]
    loss_total = loss_softmax + 0.01 * loss_center + LAMBDA * loss_triplet
    return (
        np.float32(loss_total),
        np.float32(loss_triplet),
        np.float32(loss_softmax),
        np.float32(loss_center),
    )


def kernel(anchor, positive, negative, outputs, labels_anchor, labels_neg, exemplars):
    global LAST_RESULTS
    maps, bases, W = _prep(
        anchor, positive, negative, outputs, labels_anchor, labels_neg, exemplars
    )
    nc = _build(bases, W)
    res = run_bass_kernel_spmd(nc, maps, core_ids=list(range(NCORES)))
    LAST_RESULTS = res
    return _combine(res.results)
